# revision 45
# baseline (speedup 1.0000x reference)
"""Ragged masked-attention TRN2 kernel (nn_AttentionBase, B=16 Q=K=D=1024 fp32).

Length-aware work skipping: only q-chunks < ceil(query_len/128) and k-chunks
< ceil(key_len/128) are computed; everything else in the output is exactly
zero (fully masked softmax rows) and is host-filled.

Scheduling: work units are (batch, 128-row query tile). A cost-model-guided
packer assigns units to the 8 cores (batch-affine so K/V loads are shared),
then a sim-driven local search (TimelineSim is the exact graded cost model)
rebalances units, picks per-core section order and the job sequence
(small sections can be spliced mid-stream to hide their serial softmax
chains). Each core runs a program JIT-specialized to its structural
signature; programs/sims are cached and cores with equal signatures share
one SPMD launch.

Per job (one 128-query tile against kcx = max(kc,2)*128 keys):
  scores = Q @ K^T        single-pass f32r matmuls, contraction d on
                          partitions, score tiles 256..512 wide (full PE rate)
  scores[last tile] += ones x biasrow   rank-1 matmul, 0/-1e30 per key
                          (masked key columns only exist in the final tile)
  softmax along k (free axis): negated reduce_max (DVE) -> exp(x - max) on
  ScalarE with fused row-sum -> reciprocal; q rows >= query_len zeroed via
  the per-partition output scale
  out = softmax @ V       PE-transposed bf16 weights, bf16 matmuls

All loads ride one explicitly-ordered SP HWDGE queue (q0, K d-tiles, V
interleaved with upcoming q's) so the DMA pool never starves the K stream.
f32r rounds operands to 12 significant bits at full PE rate; single-pass QK
gives ~6e-3 max rel err (gate 2e-2). W/V/out are bf16.
"""

import math
import sys

sys.path.insert(0, "/opt/trn_rl_repo")

import numpy as np
import ml_dtypes

BF16 = ml_dtypes.bfloat16

P = 128
N_CORES = 8
SEQ = 1024
D = 1024
NCH = SEQ // P  # 8 chunks along any 1024 dim
NEG = np.float32(-1e30)

_CACHE = {}
USE_BF16 = True
WARMUP_MM = 0
PS_DEPTH = 2
SMALL4 = True
RAMP4 = False
Q_ON_ACT = False
TDMA = False
LOOKAHEAD = 1
FINAL_STORE_PIECES = 2
ACT_PRELOAD = False
Q1_AFTER = 8  # load q(job1) after this many K d-tiles of section 0
QREST_AFTER = -1  # K d-index after which q0's remainder loads; -1 = unsplit q0


def _chunks(n):
    return max(1, -(-int(n) // P))


def _widths(kc):
    """Score-tile widths covering kcx = max(kc,2)*128 columns, each in
    [256, 512] so f32r matmuls run at full rate."""
    kcx = max(kc, 2) * P
    n_t = -(-kcx // 512)
    base = kcx // n_t
    ws = []
    rem = kcx
    for t in range(n_t):
        w = -(-rem // (n_t - t))
        w = -(-w // P) * P  # multiple of 128
        ws.append(w)
        rem -= w
    assert sum(ws) == kcx and all(256 <= w <= 512 for w in ws), (kc, ws)
    return ws


def _unit_pe_ns(kc):
    """PE cycles for one m-tile of a kc-chunk section, in ns."""
    kcx = max(kc, 2) * P
    return (9 * kcx + P * kc + 1024 * kc) / 2.4


def _touch_dma_ns(kc):
    """K + V DMA for touching a section (bytes/360) plus fixed overhead for
    the section transition (imperfect overlap)."""
    return (kc * P * P * NCH * 4 + kc * P * D * 2) / 360.0 + 1500.0


_UNIT_DMA_NS = (P * SEQ * 4 + P * D * 2) / 360.0  # Q f32 + out bf16 per m-tile


def _plan(query_lens, key_lens, split_factor=1.02, order_mode="ratio", dma_slack=0.0):
    """Assign (batch, m-tile) units to cores, balancing max(PE, DMA) per
    core. Returns per-core list of sections (b, m_list) plus qc/kc."""
    B = len(query_lens)
    qc = [_chunks(q) for q in query_lens]
    kc = [_chunks(k) for k in key_lens]

    unit = [_unit_pe_ns(kc[b]) for b in range(B)]
    touch = [_touch_dma_ns(kc[b]) for b in range(B)]

    pe = [0.0] * N_CORES
    dma = [0.0] * N_CORES
    cores = [[] for _ in range(N_CORES)]

    def cost(c):
        return max(pe[c], dma[c] - dma_slack)

    def cost_after(c, b, n):
        return max(pe[c] + n * unit[b], dma[c] + touch[b] + n * _UNIT_DMA_NS - dma_slack)

    total_pe = sum(qc[b] * unit[b] for b in range(B))
    total_dma = sum(qc[b] * _UNIT_DMA_NS + touch[b] for b in range(B))
    ideal = max(total_pe, total_dma) / N_CORES

    for b in sorted(range(B), key=lambda b: -(qc[b] * unit[b] + touch[b])):
        rem = qc[b]
        while rem:
            c = min(range(N_CORES), key=cost)
            # units that fit under the target on this core
            n = rem
            while n > 1 and cost_after(c, b, n) > ideal * split_factor:
                n -= 1
            merged = False
            for sj, (b2, _, n2) in enumerate(cores[c]):
                if b2 == b:
                    cores[c][sj] = (b, None, n2 + n)
                    merged = True
                    break
            if not merged:
                cores[c].append((b, None, n))  # m_list filled later
                dma[c] += touch[b]
            pe[c] += n * unit[b]
            dma[c] += n * _UNIT_DMA_NS
            rem -= n

    # local improvement: move one unit from the worst core to the best
    for _ in range(64):
        w = max(range(N_CORES), key=cost)
        candidates = []
        for si, (b, _, n) in enumerate(cores[w]):
            for c in range(N_CORES):
                if c == w:
                    continue
                extra = 0.0 if any(s[0] == b for s in cores[c]) else touch[b]
                new_w = max(pe[w] - unit[b], dma[w] - _UNIT_DMA_NS - (touch[b] if n == 1 else 0) - dma_slack)
                new_c = max(pe[c] + unit[b], dma[c] + extra + _UNIT_DMA_NS - dma_slack)
                if max(new_w, new_c) < cost(w) - 1:
                    candidates.append((max(new_w, new_c), si, c))
        if not candidates:
            break
        _, si, c = min(candidates)
        b, _, n = cores[w][si]
        if n == 1:
            cores[w].pop(si)
            dma[w] -= touch[b]
        else:
            cores[w][si] = (b, None, n - 1)
        pe[w] -= unit[b]
        dma[w] -= _UNIT_DMA_NS
        merged = False
        for sj, (b2, _, n2) in enumerate(cores[c]):
            if b2 == b:
                cores[c][sj] = (b, None, n2 + 1)
                merged = True
                break
        if not merged:
            cores[c].append((b, None, 1))
            dma[c] += touch[b]
        pe[c] += unit[b]
        dma[c] += _UNIT_DMA_NS

    # assign concrete m indices: batches split across cores get disjoint
    # contiguous runs of their m-tiles
    next_m = [0] * B
    out = [[] for _ in range(N_CORES)]
    for c in range(N_CORES):
        for b, _, n in cores[c]:
            out[c].append((b, list(range(next_m[b], next_m[b] + n))))
            next_m[b] += n
    # order sections by descending PE-per-DMA ratio: the first section's K
    # load is exposed latency either way, but a job-rich section up front
    # hides every later section's DMA under its compute; small sections
    # last also shorten the kernel tail (short PV + store).
    for c in range(N_CORES):
        if order_mode == "ratio":
            out[c].sort(
                key=lambda s: -(len(s[1]) * _unit_pe_ns(kc[s[0]]))
                / _touch_dma_ns(kc[s[0]])
            )
        elif order_mode == "kc_desc":
            out[c].sort(key=lambda s: (-kc[s[0]], -len(s[1]), s[0]))
        elif order_mode == "kc_asc":
            out[c].sort(key=lambda s: (kc[s[0]], len(s[1]), s[0]))
    return out, qc, kc


_PLAN_CANDIDATES = [
    (1.02, "ratio", 0.0),
    (1.02, "kc_desc", 0.0),
    (1.15, "ratio", 0.0),
    (1.3, "ratio", 0.0),
    (1.02, "kc_asc", 0.0),
    (1.3, "kc_desc", 0.0),
    (1.02, "ratio", 8000.0),
    (1.02, "ratio", 16000.0),
    (1.02, "ratio", 30000.0),
    (1.15, "ratio", 16000.0),
    (1.02, "kc_desc", 16000.0),
    (1.3, "ratio", 30000.0),
]
_PLAN_CACHE = {}


def _sim_sig(sig):
    from concourse.timeline_sim import TimelineSim

    t = _SIM_CACHE.get(sig)
    if t is None:
        t = TimelineSim(_get_nc(sig), trace=False).simulate()
        _SIM_CACHE[sig] = t
    return t


def _order_candidates(secs, kc):
    """Candidate in-core section orders."""
    ratio = sorted(
        secs,
        key=lambda s: -(s[1] * _unit_pe_ns(kc[s[0]])) / _touch_dma_ns(kc[s[0]]),
    )
    cands = [ratio]
    if len(secs) > 1:
        # smallest-K section first to chew on while the big K streams
        small = min(secs, key=lambda s: kc[s[0]] * 1000 + s[1])
        rest = [s for s in ratio if s is not small]
        cands.append([small] + rest)
        cands.append(list(reversed(ratio)))
        # tiny sections tucked behind the lead section: their serial softmax
        # chains hide under the lead's compute instead of the kernel tail
        tiny = [s for s in ratio if s[1] <= 2]
        if tiny and len(tiny) < len(secs):
            big = [s for s in ratio if s[1] > 2]
            cands.append(big[:1] + tiny + big[1:])
    return cands


def _mk_sig(order, kc, seq=None):
    secs = tuple((n, kc[b]) for b, n in order)
    if seq is None:
        seq = []
        for s, (n, _) in enumerate(secs):
            seq.extend([s] * n)
        seq = tuple(seq)
    return (secs, seq)


def _order_sections(secs, kc):
    """Canonical in-core order (ratio-desc) — used during plan search."""
    return _order_candidates(secs, kc)[0]


def _order_sections_best(secs, kc):
    """Best (section order, job sequence) by per-program sim (final pass)."""
    best = None
    for order in _order_candidates(secs, kc):
        for seq in _seq_candidates(order):
            t = _sim_sig(_mk_sig(order, kc, seq))
            if best is None or t < best[0]:
                best = (t, order, seq)
    return best[1], best[2]


def _core_sig(secs, kc):
    return _mk_sig(_order_sections(secs, kc), kc)


def _optimize_plan(counts, kc, rounds=120):
    """Sim-driven local search on (batch -> units per core) assignment.
    Objective: lexicographic min of sorted core times (descending) — allows
    plateau moves that shave the 2nd/3rd-worst core and unlock the max."""
    counts = [list(cs) for cs in counts]

    def core_t(cs):
        return _sim_sig(_core_sig(cs, kc)) if cs else 0.0

    times = [core_t(counts[c]) for c in range(N_CORES)]

    def obj(ts):
        return tuple(sorted(ts, reverse=True))

    for _ in range(rounds):
        order = sorted(range(N_CORES), key=lambda c: -times[c])
        cur_obj = obj(times)
        best = None
        for w in order[:3]:
            for si, (b, n) in enumerate(counts[w]):
                for dst in range(N_CORES):
                    if dst == w:
                        continue
                    for mv in ([1, n] if n > 1 else [1]):
                        new_w = [
                            (bb, nn - mv if ii == si else nn)
                            for ii, (bb, nn) in enumerate(counts[w])
                        ]
                        new_w = [(bb, nn) for bb, nn in new_w if nn > 0]
                        new_d = list(counts[dst])
                        for jj, (bb, nn) in enumerate(new_d):
                            if bb == b:
                                new_d[jj] = (bb, nn + mv)
                                break
                        else:
                            new_d.append((b, mv))
                        t_w = core_t(new_w)
                        t_d = core_t(new_d)
                        nts = list(times)
                        nts[w] = t_w
                        nts[dst] = t_d
                        no = obj(nts)
                        if no < cur_obj and (best is None or no < best[0]):
                            best = (no, w, new_w, dst, new_d)
            if best is not None:
                break
        if best is None:
            break
        _, w, new_w, dst, new_d = best
        counts[w] = new_w
        counts[dst] = new_d
        times[w] = core_t(counts[w])
        times[dst] = core_t(counts[dst])
    return counts


def _best_plan(query_lens, key_lens):
    """Candidate plans -> sim-driven local search -> lowest worst-core time."""
    key = (tuple(int(q) for q in query_lens), tuple(int(k) for k in key_lens))
    if key in _PLAN_CACHE:
        return _PLAN_CACHE[key]
    scored = []
    for sf, om, ds in _PLAN_CANDIDATES:
        cores, qc, kc = _plan(query_lens, key_lens, sf, om, ds)
        worst = max(_sim_sig(_sig(cores[c], kc)) for c in range(N_CORES))
        scored.append((worst, cores, qc, kc))
    scored.sort(key=lambda x: x[0])
    best_counts, best_worst = None, None
    qc, kc = scored[0][2], scored[0][3]
    seen_starts = set()
    for worst0, cores, _, _ in scored[:6]:
        start = tuple(
            tuple(sorted((b, len(ms)) for b, ms in cores[c])) for c in range(N_CORES)
        )
        if start in seen_starts:
            continue
        seen_starts.add(start)
        counts = [[(b, len(ms)) for b, ms in cores[c]] for c in range(N_CORES)]
        counts = _optimize_plan(counts, kc)
        worst = max(
            _sim_sig(_core_sig(counts[c], kc)) if counts[c] else 0.0
            for c in range(N_CORES)
        )
        if best_worst is None or worst < best_worst:
            best_worst, best_counts = worst, counts
    counts = best_counts
    # materialize ordered sections with concrete m indices + chosen job seq
    next_m = [0] * len(qc)
    out = []
    sigs = []
    for c in range(N_CORES):
        if not counts[c]:
            out.append([])
            sigs.append(((), ()))
            continue
        order, seq = _order_sections_best(counts[c], kc)
        lst = []
        for b, n in order:
            lst.append((b, list(range(next_m[b], next_m[b] + n))))
            next_m[b] += n
        out.append(lst)
        sigs.append(_mk_sig(order, kc, seq))
    for b in range(len(qc)):
        assert next_m[b] == qc[b], (b, next_m[b], qc[b])
    _PLAN_CACHE[key] = (out, qc, kc, sigs)
    return _PLAN_CACHE[key]


_SIM_CACHE = {}


def _seq_candidates(counts_ordered):
    """Candidate job sequences. counts_ordered: [(b, n_units)] in section
    order. Returns list of tuples of section indices."""
    seq = []
    for s, (b, n) in enumerate(counts_ordered):
        seq.extend([s] * n)
    cands = [tuple(seq)]
    if 2 <= len(counts_ordered) <= 3:
        lead_n = counts_ordered[0][1]
        tail = []
        for s, (b, n) in enumerate(counts_ordered[1:], start=1):
            tail.extend([s] * n)
        if lead_n >= 4 and 1 <= len(tail) <= lead_n - 2:
            # splice later-section jobs between lead jobs, starting after 2
            inter = [0, 0]
            k = 0
            for i in range(lead_n - 2):
                inter.append(0)
                if k < len(tail) and i >= 1:
                    inter.append(tail[k])
                    k += 1
            inter.extend(tail[k:])
            cands.append(tuple(inter))
        if lead_n >= 3 and len(tail) >= 2:
            # proportional merge, lead-first, keeping >=3 lead jobs up front
            inter2 = [0] * 3
            li, ti = 3, 0
            while li < lead_n or ti < len(tail):
                if ti < len(tail):
                    inter2.append(tail[ti]); ti += 1
                if li < lead_n:
                    inter2.append(0); li += 1
                if li < lead_n and (li - 3) * len(tail) > (ti) * (lead_n - 3):
                    inter2.append(0); li += 1
            c2 = tuple(inter2)
            if c2 not in cands:
                cands.append(c2)
    return cands


def _sig(sections, kc):
    secs = tuple((len(ms), kc[b]) for b, ms in sections)
    seq = []
    for s, (n, _) in enumerate(secs):
        seq.extend([s] * n)
    return (secs, tuple(seq))


def _build_nc(sig):
    secs, jobseq = sig
    import concourse.bass as bass  # noqa: F401
    import concourse.mybir as mybir
    import concourse.tile as tile
    from concourse import bacc
    from concourse.masks import make_identity

    f32 = mybir.dt.float32
    f32r = mybir.dt.float32r
    bf16 = mybir.dt.bfloat16 if USE_BF16 else mybir.dt.float32r
    X = mybir.AxisListType.X
    Exp = mybir.ActivationFunctionType.Exp

    nc = bacc.Bacc("TRN2", target_bir_lowering=False, debug=False)

    kt_d, v_d, bias_d, qm_d, qt_d, out_d = [], [], [], [], [], []
    for s, (n_m, kc_s) in enumerate(secs):
        kcx = max(kc_s, 2) * P
        kt_d.append(nc.dram_tensor(f"kt{s}", [NCH, P, kcx if kc_s == 1 else kc_s * P], f32r, kind="ExternalInput"))
        v_d.append(nc.dram_tensor(f"v{s}", [kc_s, P, D], bf16, kind="ExternalInput"))
        bias_d.append(
            nc.dram_tensor(f"bias{s}", [1, _widths(kc_s)[-1]], f32r, kind="ExternalInput")
        )
        qm_d.append(nc.dram_tensor(f"qm{s}", [P, n_m], f32, kind="ExternalInput"))
        qt_d.append(
            nc.dram_tensor(f"qt{s}", [n_m, NCH, P, P], f32r, kind="ExternalInput")
        )
        out_d.append(nc.dram_tensor(f"out{s}", [n_m, P, D], bf16, kind="ExternalOutput"))

    jobs = []  # (section, i_within_section) in jobseq order
    nxt = [0] * len(secs)
    for s in jobseq:
        jobs.append((s, nxt[s]))
        nxt[s] += 1
    njobs = len(jobs)
    per_sec_tags = len(secs) <= 3  # per-section SBUF tags (no WAR throttling)

    with tile.TileContext(nc) as tc:
        with (
            tc.tile_pool(name="const", bufs=1) as const_pool,
            tc.tile_pool(name="kpool", bufs=1) as kpool,
            tc.tile_pool(name="vpool", bufs=1) as vpool,
            tc.tile_pool(name="qpool", bufs=1) as qpool,
            tc.tile_pool(name="work", bufs=2) as work,
            tc.tile_pool(name="wpool", bufs=2) as wpool,
            tc.tile_pool(name="stat", bufs=2) as stat,
            tc.tile_pool(name="misc", bufs=1) as misc,
            tc.tile_pool(name="ps_s", bufs=1, space="PSUM") as ps_s,
            tc.tile_pool(name="ps_t", bufs=1, space="PSUM") as ps_t,
            tc.tile_pool(name="ps_o", bufs=1, space="PSUM") as ps_o,
        ):
            identity_f32 = const_pool.tile([P, P], f32, tag="ident32")
            make_identity(nc, identity_f32)
            identity = const_pool.tile([P, P], bf16, tag="ident")
            nc.vector.tensor_copy(identity[:], identity_f32[:])
            ones_f32 = const_pool.tile([1, P], f32, tag="ones32")
            nc.gpsimd.memset(ones_f32[:], 1.0)
            ones = const_pool.tile([1, P], f32r, tag="ones")
            nc.vector.tensor_copy(ones[:], ones_f32[:])
            if ACT_PRELOAD:
                # warm the Exp activation table during the DMA ramp so the
                # first real exp doesn't pay the 1283ns table load
                scratch = const_pool.tile([1, P], f32, tag="actwarm")
                nc.scalar.activation(scratch[:], ones_f32[:], Exp)

            kt_t = {}   # s -> [K tile [P, kcx] per d]
            vc_t = {}   # s -> [V tile [P, D] per j]
            brow_t = {}  # s -> bias row
            qm_t = {}   # s -> qmask cols

            def load_section_k(s, after_d0=None):
                n_m, kc_s = secs[s]
                kcx = max(kc_s, 2) * P
                sb = s if per_sec_tags else s % 2
                kts = []
                kw = kcx if kc_s == 1 else kc_s * P  # kc=1 ships host-padded
                tile_w = kcx if per_sec_tags else SEQ
                for d in range(NCH):
                    t = kpool.tile([P, tile_w], f32r, tag=f"k{sb}{d}", name=f"k{s}d{d}")
                    nc.sync.dma_start(t[:, :kw], kt_d[s].ap()[d])
                    kts.append(t)
                    if d == QREST_AFTER and after_d0 is not None:
                        after_d0()
                    if s == 0 and d == Q1_AFTER - 1 and njobs > 1:
                        load_q(1)
                kt_t[s] = kts
                wlast = _widths(kc_s)[-1]
                brow = misc.tile([1, wlast if per_sec_tags else SEQ], f32r, tag=f"b{sb}", name=f"b{s}")
                nc.gpsimd.dma_start(brow[:, :wlast], bias_d[s].ap()[:, :])
                brow_t[s] = brow
                qm = stat.tile([P, NCH], f32, tag=f"qm{sb}", name=f"qm{s}")
                nc.gpsimd.dma_start(qm[:, :n_m], qm_d[s].ap()[:, :])
                qm_t[s] = qm

            def load_v(s, j):
                sb = s if per_sec_tags else s % 2
                t = vpool.tile([P, D], bf16, tag=f"v{sb}{j}", name=f"v{s}j{j}")
                nc.sync.dma_start(t[:], v_d[s].ap()[j])
                vc_t.setdefault(s, []).append(t)

            qq_t = {}

            def load_q(gi, split=False):
                s, i = jobs[gi]
                t = qpool.tile([P, NCH, P], f32r, tag=f"q{gi % 4}", name=f"q{gi}")
                src_ap = qt_d[s].ap()[i].rearrange("d p c -> p d c")
                eng = nc.scalar if Q_ON_ACT else nc.sync
                if split:
                    eng.dma_start(t[:, :1], src_ap[:, :1])
                    qq_t[gi] = (t, src_ap)  # rest loaded by load_q_rest
                else:
                    eng.dma_start(t[:], src_ap)
                    qq_t[gi] = t

            def load_q_rest(gi):
                t, src_ap = qq_t[gi]
                nc.sync.dma_start(t[:, 1:], src_ap[:, 1:])
                qq_t[gi] = t

            stageb = {}

            def emit_stage_a(gi):
                s, i = jobs[gi]
                n_m, kc_s = secs[s]
                kcx = max(kc_s, 2) * P
                sb = s if per_sec_tags else s % 2
                ws = _widths(kc_s)
                T = len(ws)
                nm2 = stat.tile([P, 2], f32, tag=f"nm2{gi % 2}", name=f"nm2_{gi}")
                w_sb = wpool.tile([P, SEQ], bf16, tag=f"w{gi % 2}", name=f"w{gi}")
                rs = stat.tile([P, 2], f32, tag=f"rs{gi % 2}", name=f"rs{gi}")
                pss = []
                off = 0
                for t, w in enumerate(ws):
                    # ramp: jobs 2/3 of a leading big section borrow the idle
                    # transpose/output banks so 4 jobs' QK can dribble against
                    # the arriving K stream (PV hasn't started yet)
                    pool = ps_s
                    if RAMP4 and s == 0 and len(ws) == 2 and gi in (2, 3) and n_m >= 5:
                        pool = ps_t if gi == 2 else ps_o
                        ps = pool.tile(
                            [P, 512], f32, tag=("pst0", "pst1")[t] if gi == 2 else ("o0", "o1")[t],
                            name=f"s{gi}_{t}",
                        )
                    else:
                        depth = 3 if TDMA else 2
                        if SMALL4 and len(ws) == 1:
                            stag = gi % (2 * depth)
                        else:
                            stag = (gi % depth) * 2 + t
                        ps = ps_s.tile([P, 512], f32, tag=f"sb{stag}", name=f"s{gi}_{t}")
                    last = t == len(ws) - 1
                    for d in range(NCH):
                        nc.tensor.matmul(
                            ps[:, :w],
                            qq_t[gi][:, d],
                            kt_t[s][d][:, off : off + w],
                            start=(d == 0),
                            stop=(d == NCH - 1 and not last),
                        )
                    if last:
                        # masked key columns only exist in the final tile
                        nc.tensor.matmul(
                            ps[:, :w],
                            ones[:],
                            brow_t[s][:, :w],
                            start=False,
                            stop=True,
                        )
                    nc.vector.reduce_max(nm2[:, t : t + 1], ps[:, :w], axis=X, negate=True)
                    pss.append(ps)
                    off += w
                if T == 2:
                    negmax = stat.tile([P, 1], f32, tag=f"ngm{gi % 2}", name=f"ngm{gi}")
                    nc.vector.tensor_tensor(
                        negmax[:], nm2[:, 0:1], nm2[:, 1:2], mybir.AluOpType.min
                    )
                else:
                    negmax = nm2[:, 0:1]
                off = 0
                for t, w in enumerate(ws):
                    nc.scalar.activation(
                        w_sb[:, off : off + w],
                        pss[t][:, :w],
                        Exp,
                        bias=negmax if T == 2 else negmax,
                        accum_out=rs[:, t : t + 1],
                    )
                    off += w
                if T == 2:
                    rsum = stat.tile([P, 1], f32, tag=f"rsum{gi % 2}", name=f"rsum{gi}")
                    nc.vector.tensor_tensor(
                        rsum[:], rs[:, 0:1], rs[:, 1:2], mybir.AluOpType.add
                    )
                else:
                    rsum = rs[:, 0:1]
                rcp = stat.tile([P, 1], f32, tag=f"rcp{gi % 2}", name=f"rcp{gi}")
                nc.vector.reciprocal(rcp[:], rsum)
                scal = stat.tile([P, 1], f32, tag=f"scal{gi % 2}", name=f"scal{gi}")
                nc.vector.tensor_tensor(
                    scal[:], rcp[:], qm_t[s][:, i : i + 1], mybir.AluOpType.mult
                )
                stageb[gi] = (w_sb, scal)

            def emit_stage_b(gi):
                s, i = jobs[gi]
                n_m, kc_s = secs[s]
                sb = s if per_sec_tags else s % 2
                w_sb, scal = stageb.pop(gi)
                if TDMA:
                    wt_all = work.tile([P, NCH, P], bf16, tag=f"wta{gi % 2}", name=f"wta{gi}")
                    nc.scalar.dma_start_transpose(
                        wt_all[:, :kc_s], w_sb[:, : kc_s * P]
                    )
                    wt = [wt_all[:, j] for j in range(kc_s)]
                else:
                    wt = []
                    for j in range(kc_s):
                        pst = ps_t.tile([P, P], bf16, tag=f"pst{j % (4 - PS_DEPTH)}", name=f"pst{gi}_{j}")
                        nc.tensor.transpose(
                            pst[:], w_sb[:, j * P : (j + 1) * P], identity[:]
                        )
                        wtj = work.tile([P, P], bf16, tag=f"wt{j}", name=f"wt{gi}_{j}")
                        nc.any.tensor_copy(wtj[:], pst[:])
                        wt.append(wtj)

                out_sb = work.tile([P, D], bf16, tag=f"outsb{gi % 2}", name=f"osb{gi}")
                for n2 in range(2):
                    po = ps_o.tile([P, 512], f32, tag=f"o{n2 % (4 - PS_DEPTH)}", name=f"o{gi}_{n2}")
                    for j in range(kc_s):
                        nc.tensor.matmul(
                            po[:],
                            wt[j][:],
                            vc_t[s][j][:, n2 * 512 : (n2 + 1) * 512],
                            start=(j == 0),
                            stop=(j == kc_s - 1),
                        )
                    nc.any.tensor_scalar_mul(
                        out_sb[:, n2 * 512 : (n2 + 1) * 512], po[:], scal[:]
                    )
                if gi == njobs - 1:
                    # split final store so the tail drains in small pieces
                    npc = FINAL_STORE_PIECES
                    wpc = D // npc
                    for qtr in range(npc):
                        nc.sync.dma_start(
                            out_d[s].ap()[i][:, qtr * wpc : (qtr + 1) * wpc],
                            out_sb[:, qtr * wpc : (qtr + 1) * wpc],
                        )
                else:
                    nc.gpsimd.dma_start(out_d[s].ap()[i], out_sb[:])

            # warm the PE p-state during the DMA ramp: dummy rank-1 matmuls
            # on SBUF-resident constants (results discarded)
            if WARMUP_MM:
                warm = ps_o.tile([P, 512], f32, tag="o0", name="warm")
                for _ in range(WARMUP_MM):
                    nc.tensor.matmul(warm[:, :P], ones[:], ones_f32.bitcast(f32r)[:], start=True, stop=True)
            # ONE explicit global DMA order on the SP queue: q(job0), then per
            # section its K stream, then V interleaved with upcoming q loads.
            # The q gi%4 / section s%2 tag reuse (WAR) throttles prefetch.
            load_q(0, split=(QREST_AFTER >= 0))
            qptr = 2 if (Q1_AFTER <= 8 and njobs > 1) else 1
            sec_last = {}
            for gi, (s, i) in enumerate(jobs):
                sec_last[s] = gi
            for s in range(len(secs)):
                load_section_k(
                    s,
                    after_d0=(lambda: load_q_rest(0))
                    if (s == 0 and QREST_AFTER >= 0)
                    else None,
                )
                for j in range(secs[s][1]):
                    load_v(s, j)
                    if qptr <= sec_last[s] and qptr < njobs:
                        load_q(qptr)
                        qptr += 1
                while qptr <= sec_last[s] and qptr < njobs:
                    load_q(qptr)
                    qptr += 1
            L = min(LOOKAHEAD, max(1, njobs - 1))
            for gi in range(njobs + L):
                if gi < njobs:
                    emit_stage_a(gi)
                if gi >= L:
                    emit_stage_b(gi - L)
    nc.compile()
    return nc


def _get_nc(sig):
    if sig not in _CACHE:
        _CACHE[sig] = _build_nc(sig)
    return _CACHE[sig]


def _pack_core(sections, qc, kc, queries, keys, values, query_lens, key_lens):
    """Build the input map for one core."""
    kidx = np.arange(SEQ)
    m = {}
    for s, (b, ms) in enumerate(sections):
        kcx = max(kc[b], 2) * P
        kT = np.ascontiguousarray(keys[b].T)  # [d, k]
        ktp = kT.reshape(NCH, P, SEQ)[:, :, : kc[b] * P]
        if kc[b] == 1:  # pad to 256 cols with zeros (scores 0, masked by bias)
            ktp = np.concatenate([ktp, np.zeros((NCH, P, P), np.float32)], axis=2)
        m[f"kt{s}"] = np.ascontiguousarray(ktp)
        m[f"v{s}"] = values[b].reshape(NCH, P, D)[: kc[b]].astype(BF16 if USE_BF16 else np.float32)
        wlast = _widths(kc[b])[-1]
        bias = np.where(kidx[kcx - wlast : kcx] < key_lens[b], np.float32(0.0), NEG)
        m[f"bias{s}"] = bias.reshape(1, wlast).astype(np.float32)
        qmask = (kidx < query_lens[b]).astype(np.float32).reshape(NCH, P)
        m[f"qm{s}"] = np.ascontiguousarray(qmask[list(ms)].T)  # [P, n_m]
        qT = np.ascontiguousarray(queries[b].T)  # [d, q]
        qt_full = qT.reshape(NCH, P, NCH, P).transpose(2, 0, 1, 3)  # [m, d, p, c]
        m[f"qt{s}"] = np.ascontiguousarray(qt_full[list(ms)])
    return m


def _run(inputs, trace=False, trace_kwargs=None):
    from concourse.bass_utils import run_bass_kernel_spmd

    queries = np.asarray(inputs["queries"], dtype=np.float32)
    keys = np.asarray(inputs["keys"], dtype=np.float32)
    values = np.asarray(inputs["values"], dtype=np.float32)
    query_lens = np.asarray(inputs["query_lens"]).astype(np.int64)
    key_lens = np.asarray(inputs["key_lens"]).astype(np.int64)
    B = queries.shape[0]

    cores, qc, kc, sigs = _best_plan(query_lens, key_lens)
    in_maps = [
        _pack_core(cores[c], qc, kc, queries, keys, values, query_lens, key_lens)
        for c in range(N_CORES)
    ]

    # group cores by signature: one SPMD launch per distinct program
    groups = {}
    for c in range(N_CORES):
        groups.setdefault(sigs[c], []).append(c)

    out = np.zeros((B, SEQ, D), np.float32)
    results = [None] * N_CORES
    for sig, cs in groups.items():
        if not sig[0]:
            for c in cs:
                results[c] = {}
            continue
        nc = _get_nc(sig)
        kwargs = {}
        if trace:
            kwargs["trace"] = True
            if trace_kwargs:
                kwargs.update(trace_kwargs)
        try:
            res = run_bass_kernel_spmd(
                nc, [in_maps[c] for c in cs], core_ids=list(range(len(cs))), **kwargs
            )
        except Exception:
            import time

            time.sleep(5)
            res = run_bass_kernel_spmd(
                nc, [in_maps[c] for c in cs], core_ids=list(range(len(cs))), **kwargs
            )
        for idx, c in enumerate(cs):
            results[c] = res.results[idx]

    for c in range(N_CORES):
        for s, (b, ms) in enumerate(cores[c]):
            o = np.asarray(results[c][f"out{s}"]).astype(np.float32)
            for ii, mm in enumerate(ms):
                out[b, mm * P : (mm + 1) * P, :] = o[ii]
    return out, results


def _all_sims(inputs):
    """Per-core TimelineSim times (ns) for the programs this input requires."""
    from concourse.timeline_sim import TimelineSim

    query_lens = np.asarray(inputs["query_lens"]).astype(np.int64)
    key_lens = np.asarray(inputs["key_lens"]).astype(np.int64)
    cores, qc, kc, sigs = _best_plan(query_lens, key_lens)
    return [_sim_sig(sigs[c]) if sigs[c][0] else 0.0 for c in range(N_CORES)]


def kernel(**inputs) -> np.ndarray:
    out, _ = _run(inputs, trace=False)
    return out
